# revision 15
# baseline (speedup 1.0000x reference)
"""DocRED relation-extraction head on 8 Trainium2 NeuronCores.

Data-parallel over the batch axis: core b owns batch b's hidden_states slab
and its entity/pair indices; the classifier weights are replicated.

v6: reassociate (rel @ dense_w) @ out_w = rel @ (dense_w @ out_w), and
compute W_eff = dense_w @ out_w [2048, 98] on-device from the W stream.
The dominant 4MB dense_w stream then feeds matmuls that depend ONLY on
weights - the indirect gather, mention-sum (stage A) and entity-level
logits (eL) all run inside the stream window's PE slack, so nothing
serializes behind the gather and the PE never idles long enough for HAM
to re-throttle.

    W_eff   = per h-slab: 8 accumulating matmuls lhsT=dwT[128j,128h]
              rhs=ow[128j,98] -> PSUM [128h, 98], drained fp16 (the host
              pre-transposes dense_w per-slab so lhsT comes off the DMA)
    repT    = mention-sum of 128 gathered hidden rows via 8 matmuls
              against a block-ones matrix (fuses sum + transpose)
    eL1/eL2 = sum_hc repT[hc].T @ W_eff[hc] -> [32, 98] each; const row =
              sum_jc dense_b[jc].T @ ow[jc] + out_b
    logits[p] = eL1[head[p]] + const + eL2[tail[p]], via ONE K=65-stacked
              one-hot matmul per 128-pair tile.

Everything travels fp16 (PSUM accumulation fp32); ~7e-4 scale-relative
error vs the fp32 reference.

DMA plan: pos leads the scalar ring (its completion sem gates the gather);
slabs alternate sync/scalar; emission order keeps all 8 global HWDGE
sem-lane reuses benign (slab s reuses the lane of a DMA that completed
long before s's issue).
"""

import numpy as np
from contextlib import ExitStack

import concourse.bass as bass
import concourse.bacc as bacc
import concourse.tile as tile
import concourse.mybir as mybir
from concourse.bass_utils import run_bass_kernel_spmd

B, L, H, E, M, P, C = 8, 2048, 1024, 32, 4, 1024, 97
N_CORES = 8
HC = H // 128   # h-chunks per half / j-chunks of ow
JC = H // 128
NS = 2 * HC     # 16 dense_w slabs
PT = P // 128   # pair tiles
CP = C + 1      # class dim padded to 98 (alignment; pad column zero)

f32 = mybir.dt.float32
f16 = mybir.dt.float16
i32 = mybir.dt.int32

# A_AFTER: emit the gather-dependent stage A + eL1 after this many W_eff
# slabs; the PE reaches that point well after the gather completes, and
# the stream's DMA pacing absorbs the extra PE work.
A_AFTER = 10

# merged constant tensor "misc" column layout (all fp16, 128 partitions)
ONES0 = 0                 # [128, 32] block-ones for the mention sum
DB0 = ONES0 + E           # [128, 8] dense_b chunks
IOTA0 = DB0 + HC          # [32, 1] iota column
OB0 = IOTA0 + 1           # [1, 98] out_b on row 0 (zero padded)
OW0 = OB0 + CP            # [128, 8*98] out_w chunks (j on partitions)
MISCW = OW0 + JC * CP

_CACHE = {}


def _build():
    nc = bacc.Bacc("TRN2", target_bir_lowering=False, debug=False)

    hs = nc.dram_tensor("hs", [L, H], f16, kind="ExternalInput").ap()
    pos = nc.dram_tensor("pos", [E * M, 1], i32, kind="ExternalInput").ap()
    misc = nc.dram_tensor("misc", [128, MISCW], f16, kind="ExternalInput").ap()
    hrtr = nc.dram_tensor("hrtr", [E, 2 * P], f16, kind="ExternalInput").ap()
    # dwt: slab s columns [s*1024,(s+1)*1024) hold dw rows [s*128,(s+1)*128)
    # transposed: dwt[p, (s*8+jc)*128 + hh] = dw[s*128+hh, jc*128+p]
    dwt = nc.dram_tensor("dwt", [128, NS * H], f16, kind="ExternalInput").ap()
    out = nc.dram_tensor("out", [128, PT * C], f16, kind="ExternalOutput").ap()

    with tile.TileContext(nc) as tc, ExitStack() as ctx:
        sb = ctx.enter_context(tc.tile_pool(name="sb", bufs=1))
        wpool = ctx.enter_context(tc.tile_pool(name="w", bufs=16))
        # stream pool: W_eff ping-pong + stage A + stage D one-shot tiles
        psst = ctx.enter_context(tc.tile_pool(name="psst", bufs=4, space="PSUM"))
        # accumulator pool: long-lived bias / eL1 / eL2 groups
        psacc = ctx.enter_context(tc.tile_pool(name="psacc", bufs=4, space="PSUM"))

        # ---- pos leads the scalar ring: its completion sem gates the gather
        sb_pos = sb.tile([E * M, 1], i32)
        nc.scalar.dma_start(sb_pos[:], pos[:])
        sb_misc = sb.tile([128, MISCW], f16)
        nc.scalar.dma_start(sb_misc[:], misc[:])
        sb_hrtr = sb.tile([E, 2 * P], f16)
        nc.scalar.dma_start(sb_hrtr[:], hrtr[:])

        # ---- W slabs alternate rings: even->sync (sync leads with slab 0)
        wt = []
        for s in range(NS):
            wt.append(wpool.tile([128, H], f16, tag="wslab", name=f"wt{s}"))
        for s in range(NS):
            eng = nc.sync if s % 2 == 0 else nc.scalar
            eng.dma_start(wt[s][:], dwt[:, s * H:(s + 1) * H])

        # ---- gather the 128 mention rows of hidden_states (SWDGE lanes
        # are separate from the HWDGE lanes)
        sb_g = sb.tile([E * M, H], f16)
        nc.gpsimd.indirect_dma_start(
            out=sb_g[:],
            out_offset=None,
            in_=hs[:],
            in_offset=bass.IndirectOffsetOnAxis(ap=sb_pos[:, :1], axis=0),
        )

        # ---- const row: dense_b @ ow + out_b (weights-only, runs early)
        ps_bias = psacc.tile([1, CP], f32, tag="ps", name="bias")
        for jc in range(JC):
            nc.tensor.matmul(
                out=ps_bias[:],
                lhsT=sb_misc[:, DB0 + jc:DB0 + jc + 1],
                rhs=sb_misc[:, OW0 + jc * CP:OW0 + (jc + 1) * CP],
                start=(jc == 0), stop=(jc == JC - 1),
            )
        sb_eL = sb.tile([2 * E + 1, CP], f16)
        nc.vector.tensor_add(
            out=sb_eL[2 * E:2 * E + 1, :], in0=ps_bias[:],
            in1=sb_misc[:1, OB0:OB0 + CP])

        # ---- one-hot pair operands (DVE, early - only needs hrtr/iota)
        sb_oh = sb.tile([2 * E + 1, P], f16)
        nc.vector.tensor_tensor(
            out=sb_oh[:E, :],
            in0=sb_misc[:E, IOTA0:IOTA0 + 1].to_broadcast([E, P]),
            in1=sb_hrtr[:, :P],
            op=mybir.AluOpType.is_equal,
        )
        nc.vector.tensor_tensor(
            out=sb_oh[E:2 * E, :],
            in0=sb_misc[:E, IOTA0:IOTA0 + 1].to_broadcast([E, P]),
            in1=sb_hrtr[:, P:],
            op=mybir.AluOpType.is_equal,
        )
        nc.vector.tensor_tensor(
            out=sb_oh[2 * E:2 * E + 1, :],
            in0=sb_misc[:1, IOTA0:IOTA0 + 1].to_broadcast([1, P]),
            in1=sb_misc[:1, IOTA0:IOTA0 + 1].to_broadcast([1, P]),
            op=mybir.AluOpType.is_equal,
        )

        sb_weff = sb.tile([128, NS * CP], f16)
        sb_repT = sb.tile([128, HC * E], f16)
        ps_eL1 = psacc.tile([E, CP], f32, tag="ps", name="eL1")
        ps_eL2 = psacc.tile([E, CP], f32, tag="ps", name="eL2")

        def weff_slab(s):
            pw = psst.tile([128, CP], f32, tag="ps", name=f"pw{s}")
            for jc in range(JC):
                nc.tensor.matmul(
                    out=pw[:],
                    lhsT=wt[s][:, jc * 128:(jc + 1) * 128],
                    rhs=sb_misc[:, OW0 + jc * CP:OW0 + (jc + 1) * CP],
                    start=(jc == 0), stop=(jc == JC - 1),
                )
            nc.vector.tensor_copy(
                out=sb_weff[:, s * CP:(s + 1) * CP], in_=pw[:])

        def stage_a():
            for hc in range(HC):
                pa = psst.tile([128, E], f32, tag="ps", name=f"pa{hc}")
                nc.tensor.matmul(
                    out=pa[:],
                    lhsT=sb_g[:, hc * 128:(hc + 1) * 128],
                    rhs=sb_misc[:, ONES0:ONES0 + E],
                    start=True, stop=True,
                )
                nc.vector.tensor_copy(
                    out=sb_repT[:, hc * E:(hc + 1) * E], in_=pa[:])

        def eL_mm(eL, hc, s):
            nc.tensor.matmul(
                out=eL[:],
                lhsT=sb_repT[:, hc * E:(hc + 1) * E],
                rhs=sb_weff[:, s * CP:(s + 1) * CP],
                start=(hc == 0), stop=(hc == HC - 1),
            )

        # slabs 0..A_AFTER-1: pure W_eff; then the gather-dependent work
        # (stage A + all of eL1 + early eL2 chunks) slots into the stream's
        # DMA-wait gaps; remaining slabs interleave with their eL2 chunk.
        for s in range(A_AFTER):
            weff_slab(s)
        stage_a()
        for hc in range(HC):
            eL_mm(ps_eL1, hc, hc)
        for hc in range(A_AFTER - HC):
            eL_mm(ps_eL2, hc, HC + hc)
        for s in range(A_AFTER, NS):
            weff_slab(s)
            eL_mm(ps_eL2, s - HC, s)

        # ---- eL stack [65, 98]: rows 0-31 eL1, 32-63 eL2, row 64 = const
        nc.vector.tensor_copy(out=sb_eL[:E, :], in_=ps_eL1[:])
        nc.vector.tensor_copy(out=sb_eL[E:2 * E, :], in_=ps_eL2[:])

        # ---- stage D: stacked one-hot pair gather, one matmul per 128 pairs
        sb_out = sb.tile([128, PT * C], f16)
        for pt in range(PT):
            pl = psst.tile([128, CP], f32, tag="ps", name=f"pl{pt}")
            nc.tensor.matmul(
                out=pl[:],
                lhsT=sb_oh[:, pt * 128:(pt + 1) * 128],
                rhs=sb_eL[:],
                start=True, stop=True,
            )
            nc.vector.tensor_copy(
                out=sb_out[:, pt * C:(pt + 1) * C], in_=pl[:, :C])
            if pt == PT // 2 - 1:
                nc.scalar.dma_start(
                    out[:, :PT * C // 2], sb_out[:, :PT * C // 2])
        nc.sync.dma_start(out[:, PT * C // 2:], sb_out[:, PT * C // 2:])

    nc.compile()
    return nc


def get_compiled():
    if "nc" not in _CACHE:
        _CACHE["nc"] = _build()
    return _CACHE["nc"]


def make_in_maps(hidden_states, dense_w, dense_b, out_w, out_b,
                 entity_position_ids, head_tail_idxs):
    hidden_states = np.asarray(hidden_states)
    dense_w = np.asarray(dense_w)
    dense_b = np.asarray(dense_b)
    out_w = np.asarray(out_w)
    out_b = np.asarray(out_b)
    entity_position_ids = np.asarray(entity_position_ids)
    head_tail_idxs = np.asarray(head_tail_idxs)

    misc = np.zeros((128, MISCW), np.float16)
    misc[:, ONES0:ONES0 + E] = np.repeat(np.eye(E, dtype=np.float16), M, axis=0)
    misc[:, DB0:DB0 + HC] = (
        np.asarray(dense_b, np.float16).reshape(HC, 128).T)
    misc[:E, IOTA0] = np.arange(E, dtype=np.float16)
    misc[0, OB0:OB0 + C] = np.asarray(out_b, np.float16)  # col 97 stays 0
    owp = np.zeros((H, CP), np.float16)
    owp[:, :C] = np.asarray(out_w, np.float16)
    misc[:, OW0:] = (
        owp.reshape(JC, 128, CP).transpose(1, 0, 2).reshape(128, JC * CP))

    # dwt[p, (s*8+jc)*128 + hh] = dw[s*128+hh, jc*128+p]
    dwt = np.ascontiguousarray(
        np.asarray(dense_w, np.float16)
        .reshape(NS, 128, JC, 128)        # [s, hh, jc, p]
        .transpose(3, 0, 2, 1)            # [p, s, jc, hh]
        .reshape(128, NS * H))

    in_maps = []
    for b in range(B):
        ht = head_tail_idxs[b].astype(np.float16)  # [P, 2]
        hrtr = np.empty((E, 2 * P), np.float16)
        hrtr[:, :P] = ht[None, :, 0]
        hrtr[:, P:] = ht[None, :, 1]
        in_maps.append({
            "hs": np.ascontiguousarray(hidden_states[b], dtype=np.float16),
            "pos": np.ascontiguousarray(
                entity_position_ids[b].reshape(E * M, 1).astype(np.int32)),
            "misc": misc,
            "hrtr": hrtr,
            "dwt": dwt,
        })
    return in_maps


def kernel(hidden_states, dense_w, dense_b, out_w, out_b,
           entity_position_ids, head_tail_idxs, _trace=False, _trace_kwargs=None):
    nc = get_compiled()
    in_maps = make_in_maps(hidden_states, dense_w, dense_b, out_w, out_b,
                           entity_position_ids, head_tail_idxs)
    res = run_bass_kernel_spmd(
        nc, in_maps, core_ids=list(range(N_CORES)),
        trace=_trace, **(_trace_kwargs or {}),
    )
    outp = np.concatenate(
        [res.results[i]["out"].astype(np.float32)
         .reshape(128, PT, C).transpose(1, 0, 2).reshape(P, C)
         for i in range(N_CORES)], axis=0)
    if _trace:
        return outp, res
    return outp
